# revision 1
# baseline (speedup 1.0000x reference)
"""Trainium2 Bass kernel for the 4-layer spiking-MLP critic (T=16 IF/LIF recurrence).

Strategy
- Data-parallel over 8 NeuronCores: batch 4096 -> 512 per core; weights replicated.
- Everything runs transposed (feature dim on partitions, batch on the free dim),
  so no on-device transposes are needed anywhere.
- x @ W1.T + b1 is time-invariant: computed once into SBUF, reused all 16 steps.
- Weights and spikes are fp16; full fp32 accuracy is recovered with a hi/lo
  split: W ~= Whi + 2^-11 * Wlo (both fp16). Spikes are 0/1 (exact in fp16), so
  each layer is two fp16 matmul groups; the lo PSUM is folded in with a single
  fused scalar_tensor_tensor op ((lo * 2^-11) + hi).
- Layer-4 (non-spiking LIF, tau=2) is algebraically unrolled:
      v4_T = 2^-16 * sum_t 2^t * (s3_t @ W4.T) + (1 - 2^-16) * b4
  The weighted sum accumulates directly in a persistent PSUM bank across all 16
  steps by scaling the spike tensor with 2^t (exact in fp16), eliminating all
  per-step layer-4 elementwise work and state.
- IF membrane states carry their bias folded in (vb = v + b), saving one
  elementwise op per layer per step.
"""

import sys

sys.path.insert(0, "/opt/trn_rl_repo")

import numpy as np

P = 128
D, H, AOUT = 512, 1024, 64
N = 512           # batch per core
T = 16
KD, KH = D // P, H // P
CLO = float(2.0 ** -11)
NCORES = 8

_CACHE = {}


def _build():
    from contextlib import ExitStack
    from concourse import bacc, mybir, tile

    f32 = mybir.dt.float32
    f16 = mybir.dt.float16
    A = mybir.AluOpType
    IDENT = mybir.ActivationFunctionType.Identity

    nc = bacc.Bacc("TRN2", target_bir_lowering=False, debug=False)

    din = {}
    for name, shape, dt_ in [
        ("xh", [D, N], f16), ("xl", [D, N], f16),
        ("w1h", [D, H], f16), ("w1l", [D, H], f16),
        ("w2h", [H, H], f16), ("w2l", [H, H], f16),
        ("w3h", [H, H], f16), ("w3l", [H, H], f16),
        ("w4h", [H, AOUT], f16), ("w4l", [H, AOUT], f16),
        ("b1", [P, KH], f32), ("b2", [P, KH], f32), ("b3", [P, KH], f32),
        ("b4f", [AOUT, 1], f32),
    ]:
        din[name] = nc.dram_tensor(name, shape, dt_, kind="ExternalInput")
    dout = nc.dram_tensor("v4T", [AOUT, N], f32, kind="ExternalOutput")

    ts = lambda i, sz: slice(i * sz, (i + 1) * sz)

    with tile.TileContext(nc) as tc, ExitStack() as ctx:
        wpool = ctx.enter_context(tc.tile_pool(name="w", bufs=1))
        vpool = ctx.enter_context(tc.tile_pool(name="v", bufs=1))
        spool = ctx.enter_context(tc.tile_pool(name="s", bufs=1))
        upool = ctx.enter_context(tc.tile_pool(name="u", bufs=3))
        tpool = ctx.enter_context(tc.tile_pool(name="t", bufs=3))
        npool = ctx.enter_context(tc.tile_pool(name="n", bufs=2))
        mmps = ctx.enter_context(tc.tile_pool(name="mmps", bufs=3, space="PSUM"))
        zps = ctx.enter_context(tc.tile_pool(name="zps", bufs=1, space="PSUM"))

        def load_km(name, ko, m):
            t_ = wpool.tile([P, ko, m], f16, tag=name)
            nc.sync.dma_start(t_[:], din[name].ap().rearrange("(ko p) m -> p ko m", p=P))
            return t_

        w2h, w2l = load_km("w2h", KH, H), load_km("w2l", KH, H)
        w3h, w3l = load_km("w3h", KH, H), load_km("w3l", KH, H)
        w4h = load_km("w4h", KH, AOUT)

        b1sb = wpool.tile([P, KH], f32, tag="b1")
        nc.sync.dma_start(b1sb[:], din["b1"].ap())
        b2sb = wpool.tile([P, KH], f32, tag="b2")
        nc.sync.dma_start(b2sb[:], din["b2"].ap())
        b3sb = wpool.tile([P, KH], f32, tag="b3")
        nc.sync.dma_start(b3sb[:], din["b3"].ap())
        b4sb = wpool.tile([AOUT, 1], f32, tag="b4f")
        nc.sync.dma_start(b4sb[:], din["b4f"].ap())

        dv1 = vpool.tile([P, KH, N], f32, tag="dv1")
        v1 = vpool.tile([P, KH, N], f32, tag="v1")
        vb2 = vpool.tile([P, KH, N], f32, tag="vb2")
        vb3 = vpool.tile([P, KH, N], f32, tag="vb3")
        s1 = spool.tile([P, KH, N], f16, tag="s1")
        s2 = spool.tile([P, KH, N], f16, tag="s2")
        s3 = spool.tile([P, KH, N], f16, tag="s3")

        nc.gpsimd.memset(v1[:], 0.0)
        nc.gpsimd.memset(vb2[:], 0.0)
        nc.gpsimd.memset(vb3[:], 0.0)
        for c in range(KH):
            nc.scalar.activation(vb2[:, c, :], vb2[:, c, :], IDENT, bias=b2sb[:, ts(c, 1)])
            nc.scalar.activation(vb3[:, c, :], vb3[:, c, :], IDENT, bias=b3sb[:, ts(c, 1)])

        zh = zps.tile([AOUT, N], f32, tag="zh")

        # ---- dv1 = x @ W1.T + b1, in hi/lo pieces (x itself is split too) ----
        def _make_dv1_half(stp, xh, xl):
            def _dv1_half(half, w1h, w1l):
                for cc in range(KH // 2):
                    c = half * (KH // 2) + cc
                    ph = mmps.tile([P, N], f32, tag="ph")
                    pl = mmps.tile([P, N], f32, tag="pl")
                    for k in range(KD):
                        nc.tensor.matmul(ph[:], w1h[:, k, ts(cc, P)], xh[:, k, :],
                                         start=(k == 0), stop=(k == KD - 1))
                    for i, (wt, xt) in enumerate([(w1l, xh), (w1h, xl)]):
                        for k in range(KD):
                            nc.tensor.matmul(pl[:], wt[:, k, ts(cc, P)], xt[:, k, :],
                                             start=(i == 0 and k == 0),
                                             stop=(i == 1 and k == KD - 1))
                    tt = tpool.tile([P, N], f32, tag="t")
                    nc.vector.tensor_scalar(tt[:], pl[:], CLO, None, A.mult)
                    hh = tpool.tile([P, N], f32, tag="t")
                    nc.scalar.activation(hh[:], ph[:], IDENT, bias=b1sb[:, ts(c, 1)])
                    nc.vector.tensor_tensor(dv1[:, c, :], hh[:], tt[:], A.add)
            return _dv1_half

        with tc.tile_pool(name="startup", bufs=1) as stp:
            xh = stp.tile([P, KD, N], f16, tag="xh")
            nc.sync.dma_start(xh[:], din["xh"].ap().rearrange("(ko p) m -> p ko m", p=P))
            xl = stp.tile([P, KD, N], f16, tag="xl")
            nc.sync.dma_start(xl[:], din["xl"].ap().rearrange("(ko p) m -> p ko m", p=P))
            _dv1_half = _make_dv1_half(stp, xh, xl)
            for half in range(2):
                w1h = stp.tile([P, KD, H // 2], f16, tag="w1h")
                nc.sync.dma_start(
                    w1h[:], din["w1h"].ap().rearrange("(ko p) m -> p ko m", p=P)[:, :, ts(half, H // 2)])
                w1l = stp.tile([P, KD, H // 2], f16, tag="w1l")
                nc.sync.dma_start(
                    w1l[:], din["w1l"].ap().rearrange("(ko p) m -> p ko m", p=P)[:, :, ts(half, H // 2)])
                _dv1_half(half, w1h, w1l)

        # ---- the 16-step recurrence ----
        def if_layer(s_in, wh, wl, vb, bsb, s_out, pow2):
            for c in range(KH):
                ph = mmps.tile([P, N], f32, tag="ph")
                pl = mmps.tile([P, N], f32, tag="pl")
                for k in range(KH):
                    nc.tensor.matmul(pl[:], wl[:, k, ts(c, P)], s_in[:, k, :],
                                     start=(k == 0), stop=(k == KH - 1))
                for k in range(KH):
                    nc.tensor.matmul(ph[:], wh[:, k, ts(c, P)], s_in[:, k, :],
                                     start=(k == 0), stop=(k == KH - 1))
                tt = tpool.tile([P, N], f32, tag="t")
                nc.vector.scalar_tensor_tensor(tt[:], pl[:], CLO, vb[:, c, :], A.mult, A.add)
                u = upool.tile([P, N], f32, tag="u")
                nc.vector.tensor_tensor(u[:], ph[:], tt[:], A.add)
                if pow2 is None:
                    nc.gpsimd.tensor_scalar(s_out[:, c, :], u[:], 1.0, None, A.is_ge)
                else:
                    nc.vector.tensor_scalar(s_out[:, c, :], u[:], 1.0, pow2, A.is_ge, A.mult)
                nn = npool.tile([P, N], f16, tag="n")
                nc.gpsimd.tensor_scalar(nn[:], u[:], 1.0, None, A.is_lt)
                t2 = tpool.tile([P, N], f32, tag="t2")
                nc.gpsimd.tensor_tensor(t2[:], u[:], nn[:], A.mult)
                nc.scalar.activation(vb[:, c, :], t2[:], IDENT, bias=bsb[:, ts(c, 1)])

        for t in range(T):
            # layer 1: dv1 is constant; pure elementwise
            for c in range(KH):
                u = upool.tile([P, N], f32, tag="u")
                nc.vector.tensor_tensor(u[:], dv1[:, c, :], v1[:, c, :], A.add)
                nc.gpsimd.tensor_scalar(s1[:, c, :], u[:], 1.0, None, A.is_ge)
                nn = npool.tile([P, N], f16, tag="n")
                nc.gpsimd.tensor_scalar(nn[:], u[:], 1.0, None, A.is_lt)
                nc.vector.tensor_tensor(v1[:, c, :], u[:], nn[:], A.mult)

            if_layer(s1, w2h, w2l, vb2, b2sb, s2, None)
            if_layer(s2, w3h, w3l, vb3, b3sb, s3, float(2.0 ** t))

            for k in range(KH):
                nc.tensor.matmul(zh[:], w4h[:, k, :], s3[:, k, :],
                                 start=(t == 0 and k == 0), stop=(t == T - 1 and k == KH - 1),
                                 skip_group_check=True)

        fout = tpool.tile([AOUT, N], f32, tag="fout")
        nc.scalar.activation(fout[:], zh[:], IDENT, scale=float(2.0 ** -T), bias=b4sb[:])
        nc.sync.dma_start(dout.ap(), fout[:])

    nc.compile()
    return nc


def _hilo(a):
    hi = a.astype(np.float16)
    lo = ((a.astype(np.float32) - hi.astype(np.float32)) * np.float32(2.0 ** 11)).astype(np.float16)
    return hi, lo


def _prep_inputs(x, W1, b1, W2, b2, W3, b3, W4, b4):
    xT = np.ascontiguousarray(x.T.astype(np.float32))          # (D, B)
    xh, xl = _hilo(xT)
    w1h, w1l = _hilo(np.ascontiguousarray(W1.T))               # (D, H)
    w2h, w2l = _hilo(np.ascontiguousarray(W2.T))               # (H, H)
    w3h, w3l = _hilo(np.ascontiguousarray(W3.T))
    w4h, w4l = _hilo(np.ascontiguousarray(W4.T))               # (H, AOUT)
    shared = {
        "w1h": w1h, "w1l": w1l, "w2h": w2h, "w2l": w2l,
        "w3h": w3h, "w3l": w3l, "w4h": w4h, "w4l": w4l,
        "b1": np.ascontiguousarray(b1.reshape(KH, P).T.astype(np.float32)),
        "b2": np.ascontiguousarray(b2.reshape(KH, P).T.astype(np.float32)),
        "b3": np.ascontiguousarray(b3.reshape(KH, P).T.astype(np.float32)),
        "b4f": ((1.0 - 2.0 ** -T) * b4).astype(np.float32).reshape(AOUT, 1),
    }
    in_maps = []
    for i in range(NCORES):
        m = dict(shared)
        m["xh"] = np.ascontiguousarray(xh[:, i * N:(i + 1) * N])
        m["xl"] = np.ascontiguousarray(xl[:, i * N:(i + 1) * N])
        in_maps.append(m)
    return in_maps


def _run(in_maps):
    from concourse.bass_utils import run_bass_kernel_spmd
    if "nc" not in _CACHE:
        _CACHE["nc"] = _build()
    res = run_bass_kernel_spmd(_CACHE["nc"], in_maps, list(range(NCORES)))
    parts = [res.results[i]["v4T"] for i in range(NCORES)]     # each (AOUT, N)
    return np.ascontiguousarray(np.concatenate(parts, axis=1).T).astype(np.float32)


def kernel(x, W1, b1, W2, b2, W3, b3, W4, b4):
    in_maps = _prep_inputs(x, W1, b1, W2, b2, W3, b3, W4, b4)
    return _run(in_maps)



# revision 2
# speedup vs baseline: 1.0102x; 1.0102x over previous
"""Trainium2 Bass kernel for the 4-layer spiking-MLP critic (T=16 IF/LIF recurrence).

v2 strategy (vs baseline):
- Same precision scheme (fp16 hi/lo ~ 2^-22 weights; W4 hi-only) -- measured to be
  the floor that keeps rel_err ~1.2e-2 under the chaotic spike dynamics.
- Merged PSUM groups: the lo matmuls use 2^-11-pre-scaled spike tensors so hi+lo
  accumulate into ONE bank -> no per-tile fold op, half the PSUM groups.
- Lean elementwise: 4-5 ops/tile (u, s, [s_scaled], t2, bias) balanced across
  DVE/Pool/ACT so they hide completely under the PE stream.
- PE never idles: within a step, L2 c-groups stream back-to-back; L3 starts right
  after L2 (its k=7 operand arrives during the first group); L4's k=7 matmul is
  delayed past the next step's first L2 group to cover the s3[7] latency; L1(t+1)
  elementwise runs during the L3(t)/L4(t) window. Avoiding gaps also avoids the
  PE pstate ramp-down (2x slower matmuls after any idle until 3us continuous).
- dv1 = x @ W1.T + b1 computed once in three merged hi/lo groups, bias folded in.
"""

import sys

sys.path.insert(0, "/opt/trn_rl_repo")

import numpy as np
import ml_dtypes

P = 128
D, H, AOUT = 512, 1024, 64
N = 512           # batch per core
T = 16
KD, KH = D // P, H // P
CLO = float(2.0 ** -11)
NCORES = 8

_CACHE = {}


def _build():
    from contextlib import ExitStack
    from concourse import bacc, mybir, tile

    f32 = mybir.dt.float32
    f16 = mybir.dt.float16
    f8 = mybir.dt.float8e4
    DR = mybir.MatmulPerfMode.DoubleRow
    A = mybir.AluOpType
    IDENT = mybir.ActivationFunctionType.Identity

    nc = bacc.Bacc("TRN2", target_bir_lowering=False, debug=False)

    din = {}
    for name, shape, dt_ in [
        ("xh", [D, N], f16), ("xhs", [D, N], f16), ("xlr", [D, N], f16),
        ("w1h", [D, H], f16), ("w1l", [D, H], f16),
        ("w2h", [H, H], f16), ("w2l", [H, H], f16),
        ("w3h", [H, H], f16),
        ("w4h", [H, AOUT], f16), ("w3l8", [H, H], f8),
        ("b1", [P, KH], f32), ("b2", [P, KH], f32), ("b3", [P, KH], f32),
        ("b4f", [AOUT, 1], f32),
    ]:
        din[name] = nc.dram_tensor(name, shape, dt_, kind="ExternalInput")
    dout = nc.dram_tensor("v4T", [AOUT, N], f32, kind="ExternalOutput")

    ts = lambda i, sz: slice(i * sz, (i + 1) * sz)

    with tile.TileContext(nc) as tc, ExitStack() as ctx:
        wpool = ctx.enter_context(tc.tile_pool(name="w", bufs=1))
        vpool = ctx.enter_context(tc.tile_pool(name="v", bufs=1))
        spool = ctx.enter_context(tc.tile_pool(name="s", bufs=1))
        upool = ctx.enter_context(tc.tile_pool(name="u", bufs=2))
        tpool = ctx.enter_context(tc.tile_pool(name="t", bufs=2))
        mmps = ctx.enter_context(tc.tile_pool(name="mmps", bufs=4, space="PSUM"))
        plps = ctx.enter_context(tc.tile_pool(name="plps", bufs=2, space="PSUM"))
        zps = ctx.enter_context(tc.tile_pool(name="zps", bufs=1, space="PSUM"))

        # DMA order matters: everything the dv1 startup phase needs goes first
        # (x pieces, W1 pieces, biases), then W2/W4; W3 is loaded after the
        # startup pool frees its space.
        stp_ctx = tc.tile_pool(name="startup", bufs=1)
        stp = stp_ctx.__enter__()
        xh = stp.tile([P, KD, N], f16, tag="xh")
        nc.sync.dma_start(xh[:], din["xh"].ap().rearrange("(ko p) m -> p ko m", p=P))
        w1h = stp.tile([P, KD, H], f16, tag="w1h")
        nc.sync.dma_start(w1h[:], din["w1h"].ap().rearrange("(ko p) m -> p ko m", p=P))
        xhs = stp.tile([P, KD, N], f16, tag="xhs")
        nc.sync.dma_start(xhs[:], din["xhs"].ap().rearrange("(ko p) m -> p ko m", p=P))
        xlr = stp.tile([P, KD, N], f16, tag="xlr")
        nc.sync.dma_start(xlr[:], din["xlr"].ap().rearrange("(ko p) m -> p ko m", p=P))
        b1sb = wpool.tile([P, KH], f32, tag="b1")
        nc.sync.dma_start(b1sb[:], din["b1"].ap())
        b2sb = wpool.tile([P, KH], f32, tag="b2")
        nc.sync.dma_start(b2sb[:], din["b2"].ap())
        b3sb = wpool.tile([P, KH], f32, tag="b3")
        nc.sync.dma_start(b3sb[:], din["b3"].ap())
        b4sb = wpool.tile([AOUT, 1], f32, tag="b4f")
        nc.sync.dma_start(b4sb[:], din["b4f"].ap())
        w1l = stp.tile([P, KD, H], f16, tag="w1l")
        nc.sync.dma_start(w1l[:], din["w1l"].ap().rearrange("(ko p) m -> p ko m", p=P))

        def load_km(name, ko, m):
            t_ = wpool.tile([P, ko, m], f16, tag=name)
            nc.sync.dma_start(t_[:], din[name].ap().rearrange("(ko p) m -> p ko m", p=P))
            return t_

        w2h, w2l = load_km("w2h", KH, H), load_km("w2l", KH, H)
        w4h = load_km("w4h", KH, AOUT)

        dv1 = vpool.tile([P, KH, N], f32, tag="dv1")     # pre-biased: x@W1.T + b1
        v1 = vpool.tile([P, KH, N], f32, tag="v1")
        vb2 = vpool.tile([P, KH, N], f32, tag="vb2")     # v2 + b2
        vb3 = vpool.tile([P, KH, N], f32, tag="vb3")     # v3 + b3
        # s1/s1s are double-buffered so L1(t+1) can run during the L2(t)
        # window without a WAR hazard against L2(t)'s matmul reads
        s1_0 = spool.tile([P, KH, N], f16, tag="s1_0")
        s1_1 = spool.tile([P, KH, N], f16, tag="s1_1")
        s1s_0 = spool.tile([P, KH, N], f16, tag="s1s_0")
        s1s_1 = spool.tile([P, KH, N], f16, tag="s1s_1")
        s1b, s1sb = [s1_0, s1_1], [s1s_0, s1s_1]
        s2 = spool.tile([P, KH, N], f16, tag="s2")
        s28 = spool.tile([P, KH // 2, 2, N], f8, tag="s28")
        s3 = spool.tile([P, KH, N], f16, tag="s3")       # s3 * 2^t

        zh = zps.tile([AOUT, N], f32, tag="zh")

        # ---- dv1 = x @ W1.T + b1: one merged psum group per output tile ----
        # psum = W1h@xh + W1h@(x - fp16(x)) + (W1l*2^11)@(xh*2^-11)
        # (the w1l-dependent part goes last: its DMA lands latest)
        for c in range(KH):
            ps = mmps.tile([P, N], f32, tag="ps")
            nmm = 3 * KD
            i = 0
            for wt, xt in [(w1h, xh), (w1h, xlr), (w1l, xhs)]:
                for k in range(KD):
                    nc.tensor.matmul(ps[:], wt[:, k, ts(c, P)], xt[:, k, :],
                                     start=(i == 0), stop=(i == nmm - 1))
                    i += 1
            nc.scalar.activation(dv1[:, c, :], ps[:], IDENT, bias=b1sb[:, ts(c, 1)])
        stp_ctx.__exit__(None, None, None)

        # ---- L1 elementwise tile-chain for step t (s1, s1s, v1 update) ----
        # the 2^-11-scaled spike copy runs on the otherwise-idle ACT engine
        def l1_tile(t, c):
            s1, s1s = s1b[t % 2], s1sb[t % 2]
            if t == 0:
                u = dv1[:, c, :]
            else:
                ut = upool.tile([P, N], f32, tag="u1")
                nc.vector.tensor_tensor(ut[:], dv1[:, c, :], v1[:, c, :], A.add)
                u = ut[:]
            nc.gpsimd.tensor_scalar(s1[:, c, :], u, 1.0, None, A.is_ge)
            nc.scalar.activation(s1s[:, c, :], s1[:, c, :], IDENT, scale=CLO)
            nc.vector.scalar_tensor_tensor(v1[:, c, :], u, 1.0, u, A.is_lt, A.mult)

        def l1_elem(t):
            for c in range(KH):
                l1_tile(t, c)

        # ---- one IF layer tile-chain (L2/L3) ----
        # pl8: optional separate fp8-lo psum bank, folded in with scale 2^-21
        def if_tile(ps, vb, bsb, c, s_out, ss_out, pow2, pl8=None, s8_out=None):
            u = upool.tile([P, N], f32, tag="u")
            # the spike-producing chain gets scheduler priority: downstream
            # matmuls stall on it, while t2/vb' are only needed next step
            with tc.high_priority(offset=600):
                if pl8 is not None:
                    tf = tpool.tile([P, N], f32, tag="tf")
                    nc.vector.tensor_tensor(tf[:], ps[:], vb[:, c, :], A.add)
                    nc.vector.scalar_tensor_tensor(u[:], pl8[:], float(2.0 ** -21),
                                                   tf[:], A.mult, A.add)
                else:
                    nc.vector.tensor_tensor(u[:], ps[:], vb[:, c, :], A.add)
                if pow2 is None:
                    nc.gpsimd.tensor_scalar(s_out[:, c, :], u[:], 1.0, None, A.is_ge)
                else:
                    nc.vector.tensor_scalar(s_out[:, c, :], u[:], 1.0, pow2, A.is_ge, A.mult)
                if ss_out is not None:
                    nc.scalar.activation(ss_out[:, c, :], s_out[:, c, :], IDENT, scale=CLO)
                if s8_out is not None:
                    nc.vector.tensor_scalar(s8_out[:, c // 2, c % 2, :], u[:], 1.0, None, A.is_ge)
            t2 = tpool.tile([P, N], f32, tag="t2")
            nc.vector.scalar_tensor_tensor(t2[:], u[:], 1.0, u[:], A.is_lt, A.mult)
            nc.scalar.activation(vb[:, c, :], t2[:], IDENT, bias=bsb[:, ts(c, 1)])

        l1_elem(0)

        nc.gpsimd.memset(vb2[:], 0.0)
        nc.gpsimd.memset(vb3[:], 0.0)
        for c in range(KH):
            nc.scalar.activation(vb2[:, c, :], vb2[:, c, :], IDENT, bias=b2sb[:, ts(c, 1)])
            nc.scalar.activation(vb3[:, c, :], vb3[:, c, :], IDENT, bias=b3sb[:, ts(c, 1)])

        # W3 is loaded into the space freed by the startup pool (it is not
        # needed until ~27us into step 0, well after its DMA completes)
        w3pool = ctx.enter_context(tc.tile_pool(name="w3", bufs=1))
        w3h = w3pool.tile([P, KH, H], f16, tag="w3h")
        nc.sync.dma_start(w3h[:], din["w3h"].ap().rearrange("(ko p) m -> p ko m", p=P))
        w3l8 = w3pool.tile([P, KH // 2, 2, H], f8, tag="w3l8")
        nc.sync.dma_start(
            w3l8[:], din["w3l8"].ap().rearrange("(kk two p) m -> p kk two m", p=P, two=2))

        def l2_group(t, c, late7=False):
            ps = mmps.tile([P, N], f32, tag="ps")
            pairs = [(wt, st, k) for wt, st in [(w2h, s1b[t % 2]), (w2l, s1sb[t % 2])]
                     for k in range(KH)]
            if late7:
                pairs = [q for q in pairs if q[2] != KH - 1] +                         [q for q in pairs if q[2] == KH - 1]
            for i, (wt, st, k) in enumerate(pairs):
                nc.tensor.matmul(ps[:], wt[:, k, ts(c, P)], st[:, k, :],
                                 start=(i == 0), stop=(i == 15))
            if_tile(ps, vb2, b2sb, c, s2, None, None, s8_out=s28)

        # ---- the 16-step recurrence ----
        zh_started = False

        def l4_prev(k):
            if k < KH:
                nc.tensor.matmul(zh[:], w4h[:, k, :], s3[:, k, :],
                                 start=False, stop=False, skip_group_check=True)

        for t in range(T):
            # L2: 8 merged-group tiles (c=0 of steps t>0 was emitted early,
            # at the end of the previous step, to cover the L4 k=7 latency).
            # L1(t+1) tile-chains interleave into this window, one per group,
            # keeping the Pool/DVE queues smoothly fed.
            for c in range(1 if t > 0 else 0, KH):
                l2_group(t, c, late7=(t == 0 and c == 0))
                if t > 0 and c <= 4:
                    l4_prev(c + 3)   # L4 k4..7 of step t-1, now safely ready
                if t < T - 1 and c >= 1:
                    l1_tile(t + 1, c - 1)

            if t < T - 1:
                l1_tile(t + 1, KH - 1)   # last L1 tile, right after the L2 window

            # L3: hi = 8 fp16 matmuls, lo = 4 fp8 DoubleRow matmuls (K=256 each)
            def l4_mm(k, last=False):
                nonlocal zh_started
                nc.tensor.matmul(zh[:], w4h[:, k, :], s3[:, k, :],
                                 start=(not zh_started), stop=last,
                                 skip_group_check=True)
                zh_started = True

            for c in range(KH):
                ps = mmps.tile([P, N], f32, tag="ps")
                pl = plps.tile([P, N], f32, tag="pl")
                if c == 0:
                    for k in range(KH - 1):
                        nc.tensor.matmul(ps[:], w3h[:, k, ts(c, P)], s2[:, k, :],
                                         start=(k == 0), stop=False,
                                         skip_group_check=True)
                    for kk in range(KH // 2):
                        nc.tensor.matmul(pl[:], w3l8[:, kk, :, ts(c, P)], s28[:, kk, :, :],
                                         start=(kk == 0), stop=(kk == KH // 2 - 1),
                                         perf_mode=DR)
                    nc.tensor.matmul(ps[:], w3h[:, KH - 1, ts(c, P)], s2[:, KH - 1, :],
                                     start=False, stop=True, skip_group_check=True)
                else:
                    for k in range(KH):
                        nc.tensor.matmul(ps[:], w3h[:, k, ts(c, P)], s2[:, k, :],
                                         start=(k == 0), stop=(k == KH - 1))
                    for kk in range(KH // 2):
                        nc.tensor.matmul(pl[:], w3l8[:, kk, :, ts(c, P)], s28[:, kk, :, :],
                                         start=(kk == 0), stop=(kk == KH // 2 - 1),
                                         perf_mode=DR)
                if_tile(ps, vb3, b3sb, c, s3, None, float(2.0 ** t), pl8=pl)
                if c >= 5:
                    l4_mm(c - 5)   # L4 k0..2 ride along once their s3 is ready
                if t == T - 1 and c == KH - 1:
                    l4_mm(3)

            # L4 k3..7: k3 now, the rest woven into the next step's L2
            # groups (or the tail at t == T-1)
            if t < T - 1:
                l4_mm(3)
            if t < T - 1:
                l2_group(t + 1, 0)
                pend_l4 = list(range(4, KH))
            else:
                for k in range(4, KH):
                    l4_mm(k, last=(k == KH - 1))

        fout = tpool.tile([AOUT, N], f32, tag="fout")
        nc.scalar.activation(fout[:], zh[:], IDENT, scale=float(2.0 ** -T), bias=b4sb[:])
        nc.sync.dma_start(dout.ap(), fout[:])

    nc.compile()
    return nc


def _hilo(a):
    hi = a.astype(np.float16)
    lo = ((a.astype(np.float32) - hi.astype(np.float32)) * np.float32(2.0 ** 11)).astype(np.float16)
    return hi, lo


def _prep_inputs(x, W1, b1, W2, b2, W3, b3, W4, b4):
    xT = np.ascontiguousarray(x.T.astype(np.float32))          # (D, B)
    xh = xT.astype(np.float16)
    xlr = (xT - xh.astype(np.float32)).astype(np.float16)      # raw residual
    xhs = (xh.astype(np.float32) * np.float32(2.0 ** -11)).astype(np.float16)
    w1h, w1l = _hilo(np.ascontiguousarray(W1.T))               # (D, H)
    w2h, w2l = _hilo(np.ascontiguousarray(W2.T))               # (H, H)
    w3T = np.ascontiguousarray(W3.T).astype(np.float32)
    w3h = w3T.astype(np.float16)
    w3l8 = ((w3T - w3h.astype(np.float32)) * np.float32(2.0 ** 21)).astype(
        ml_dtypes.float8_e4m3)
    w4h = np.ascontiguousarray(W4.T).astype(np.float16)        # (H, AOUT)
    shared = {
        "w1h": w1h, "w1l": w1l, "w2h": w2h, "w2l": w2l,
        "w3h": w3h, "w3l8": w3l8, "w4h": w4h,
        "b1": np.ascontiguousarray(b1.reshape(KH, P).T.astype(np.float32)),
        "b2": np.ascontiguousarray(b2.reshape(KH, P).T.astype(np.float32)),
        "b3": np.ascontiguousarray(b3.reshape(KH, P).T.astype(np.float32)),
        "b4f": ((1.0 - 2.0 ** -T) * b4).astype(np.float32).reshape(AOUT, 1),
    }
    in_maps = []
    for i in range(NCORES):
        m = dict(shared)
        m["xh"] = np.ascontiguousarray(xh[:, i * N:(i + 1) * N])
        m["xhs"] = np.ascontiguousarray(xhs[:, i * N:(i + 1) * N])
        m["xlr"] = np.ascontiguousarray(xlr[:, i * N:(i + 1) * N])
        in_maps.append(m)
    return in_maps


def _run(in_maps):
    from concourse.bass_utils import run_bass_kernel_spmd
    if "nc" not in _CACHE:
        _CACHE["nc"] = _build()
    res = run_bass_kernel_spmd(_CACHE["nc"], in_maps, list(range(NCORES)))
    parts = [res.results[i]["v4T"] for i in range(NCORES)]     # each (AOUT, N)
    return np.ascontiguousarray(np.concatenate(parts, axis=1).T).astype(np.float32)


def kernel(x, W1, b1, W2, b2, W3, b3, W4, b4):
    in_maps = _prep_inputs(x, W1, b1, W2, b2, W3, b3, W4, b4)
    return _run(in_maps)
